# revision 11
# baseline (speedup 1.0000x reference)
"""Trainium2 Bass kernel for single-head attention with softmax over the query axis.

Reference computation (B=4, S=4096, DIM=768, D=96):
    q = x @ Wq + bq; k = x @ Wk + bk; v = x @ Wv + bv        # [B,S,D]
    att = einsum('bqd,bkd->bqk', q, k) / sqrt(D)             # [B,Sq,Sk]
    p   = softmax(att, axis=1)                               # over the QUERY axis
    out = einsum('bqk,bkd->bqd', p, v)

Sharding: 8 cores = 4 batches x 2 key-halves. Softmax over q is local to a
key-shard (it normalizes each key-column over all queries), and the output
contraction over k is a sum over the two key-halves, done host-side.

SPMD uniformity trick: every core runs the identical program "K/V come from
rows 0:2048 of my x, Q from all 4096 rows". The host hands core (b, kh=1) a
row-rolled copy of x[b] so its key half lands in rows 0:2048; softmax over q
is permutation-invariant, and the host un-rolls that core's partial output.

Host precomputation (legal data prep inside kernel()): x is rolled,
transposed to xT [768, 4096] and cast to fp16; Wq/bq are pre-scaled by
1/sqrt(D) so no separate score scaling is needed; weights pre-cast to fp16.

On-device (per core):
  xT  [768, 4096] fp16 in SBUF (12 KB/partition)
  QT = Wq^T xT  [96, 4096], KT/VT likewise for rows 0:2048      (fp16)
  V[kk]  [128, 96] = PE-transpose of VT 128-column blocks        (fp16)
  scoresT[128k, q] = KT_kk^T QT in PSUM; exp on scalar engine with fused
      row-sum (accum_out); no max-subtraction (scores bounded ~|7.3|)
  Vs[kk] = V[kk] * (1/rowsum_kk)  folds softmax normalization into V
  outT[96, 4096] += Vs_kk^T @ expT_kk, accumulated in PSUM over kk;
      PV for q-blocks 0:2048 is software-pipelined inside the scores/exp
      loop (PSUM: 4 banks scores + 4 banks PV), the rest follows after.
"""

import os
import sys

import numpy as np

for _p in ("/opt/trn_rl_repo",):
    if _p not in sys.path and os.path.isdir(_p):
        sys.path.insert(0, _p)

B, S, DIM, D = 4, 4096, 768, 96
SK = S // 2          # local keys per core
N_CORES = 8
NDC = DIM // 128     # 6 dim chunks
NKK = SK // 128      # 16 local key chunks

_CACHE = {}


def _build_module():
    import concourse.bass as bass
    import concourse.tile as tile
    from concourse import bacc, mybir
    from concourse.masks import make_identity
    from concourse.tile import add_dep_helper

    fp32 = mybir.dt.float32
    fp16 = mybir.dt.float16

    nc = bacc.Bacc("TRN2", target_bir_lowering=False, debug=False,
                   num_devices=N_CORES)

    xT_ap = nc.dram_tensor("xT", [DIM, S], fp16, kind="ExternalInput").ap()
    wq_ap = nc.dram_tensor("wq", [DIM, D], fp16, kind="ExternalInput").ap()
    wk_ap = nc.dram_tensor("wk", [DIM, D], fp16, kind="ExternalInput").ap()
    wv_ap = nc.dram_tensor("wv", [DIM, D], fp16, kind="ExternalInput").ap()
    bq_ap = nc.dram_tensor("bq", [D, 1], fp32, kind="ExternalInput").ap()
    bk_ap = nc.dram_tensor("bk", [D, 1], fp32, kind="ExternalInput").ap()
    bv_ap = nc.dram_tensor("bv", [D, 1], fp32, kind="ExternalInput").ap()
    outT_ap = nc.dram_tensor("outT", [D, S], fp16, kind="ExternalOutput").ap()

    with tile.TileContext(nc) as tc:
        with (
            tc.tile_pool(name="singles", bufs=1) as singles,
            tc.tile_pool(name="acts", bufs=1) as acts,
            tc.tile_pool(name="outp", bufs=4) as outp,
        ):
            # Weights/biases first (tiny), then xT halves chained two-deep
            # so early chunks finish early (concurrent DMAs share bandwidth
            # equally; unchained, the first chunk lands no earlier than the
            # last and compute can't start for ~17us).
            w_sb = {}
            for name, ap in (("q", wq_ap), ("k", wk_ap), ("v", wv_ap)):
                w = singles.tile([128, NDC, D], fp16, tag=f"w{name}")
                nc.sync.dma_start(w[:], ap.rearrange("(c p) j -> p c j", p=128))
                w_sb[name] = w
            b_sb = {}
            for name, ap in (("q", bq_ap), ("k", bk_ap), ("v", bv_ap)):
                t = singles.tile([D, 1], fp32, tag=f"b{name}")
                nc.sync.dma_start(t[:], ap[:])
                b_sb[name] = t
            xTs = []
            for dc in range(NDC):
                t = singles.tile([128, S], fp16, tag=f"xT{dc}",
                                 name=f"xT{dc}")
                xTs.append(t)
            chain = []
            for half in range(2):
                lo, hi = (0, SK) if half == 0 else (SK, S)
                for dc in range(NDC):
                    h = nc.sync.dma_start(xTs[dc][:, lo:hi],
                                          xT_ap[dc * 128:(dc + 1) * 128,
                                                lo:hi])
                    chain.append(h)
            for i in range(2, len(chain)):
                add_dep_helper(chain[i].ins, chain[i - 2].ins,
                               reason="stagger xT DMAs: finish in issue order")
            identity = singles.tile([128, 128], fp16)
            make_identity(nc, identity[:])

            QT = acts.tile([D, S], fp16, tag="QT")
            KT = acts.tile([D, SK], fp16, tag="KT")
            VT = acts.tile([D, SK], fp16, tag="VT")
            V = acts.tile([128, NKK, D], fp16, tag="V")
            Vs = acts.tile([128, NKK, D], fp16, tag="Vs")
            sums = acts.tile([128, NKK, 4], fp32, tag="sums")
            rsum = acts.tile([128, NKK], fp32, tag="rsum")
            rrec = acts.tile([128, NKK], fp32, tag="rrec")
            # exp(scores) for q 2048:4096 stays resident for the trailing PV
            # pass; q 0:2048 lives in a small rotating pool consumed by the
            # software-pipelined PV inside the scores loop.
            expT_hi = acts.tile([128, NKK, S // 2], fp16, tag="expT_hi")

            # ---------------- Phase P: projections -----------------------
            # Sequential passes with a 4-slot PSUM accumulator rotation so
            # the scores PSUM pools can open early later if needed. Pass
            # order follows DMA arrival: lo halves of xT feed QT(q 0:2048),
            # KT and VT; the hi halves only feed QT(q 2048:4096).
            with tc.tile_pool(name="ps_proj", bufs=4, space="PSUM") as ps_proj:

                def proj_pass(dst, wname, sb_base, nsb, drain_bias):
                    accs = [ps_proj.tile([D, 512], fp32, tag="pp",
                                         name=f"acc_{wname}_{sb_base}_{i}")
                            for i in range(nsb)]
                    for dc in range(NDC):
                        for i in range(nsb):
                            sb = sb_base + i
                            nc.tensor.matmul(
                                accs[i][:], w_sb[wname][:, dc, :],
                                xTs[dc][:, sb * 512:(sb + 1) * 512],
                                start=(dc == 0), stop=(dc == NDC - 1))
                    for i in range(nsb):
                        sb = sb_base + i
                        nc.vector.tensor_scalar_add(
                            dst[:, sb * 512:(sb + 1) * 512],
                            accs[i][:], drain_bias[:])

                proj_pass(QT, "q", 0, 4, b_sb["q"])
                proj_pass(KT, "k", 0, 4, b_sb["k"])
                proj_pass(VT, "v", 0, 4, b_sb["v"])
                proj_pass(QT, "q", 4, 4, b_sb["q"])

                # V[kk] = transpose(VT 128-col blocks) -> [128, 96]
                with tc.tile_pool(name="ps_t", bufs=2, space="PSUM") as ps_t:
                    for kk in range(NKK):
                        pt = ps_t.tile([128, D], fp16, tag="pt")
                        nc.tensor.transpose(
                            pt[:], VT[:, kk * 128:(kk + 1) * 128],
                            identity[:D, :D])
                        nc.vector.tensor_copy(V[:, kk, :], pt[:])

            # ------------- Phase S: scores/exp + pipelined PV(qb 0..3) ----
            def pv_matmuls(kk, po, src_tile, src_off):
                for qb in range(4):
                    nc.tensor.matmul(
                        po[qb][:], Vs[:, kk, :],
                        src_tile[:, src_off + qb * 512:
                                 src_off + (qb + 1) * 512],
                        start=(kk == 0), stop=(kk == NKK - 1))

            def drain_po(po, qb_base):
                for qb in range(4):
                    ob = outp.tile([D, 512], fp16, tag="ob")
                    nc.vector.tensor_copy(ob[:], po[qb][:])
                    nc.sync.dma_start(
                        outT_ap[:, (qb_base + qb) * 512:
                                (qb_base + qb + 1) * 512], ob[:])

            with (
                tc.tile_pool(name="ps_o1", bufs=4, space="PSUM") as ps_o1,
                tc.tile_pool(name="explo", bufs=2) as explo_pool,
            ):
                po1 = [ps_o1.tile([D, 512], fp32, tag="po", name=f"po1_{i}") for i in range(4)]
                prev_lo = None
                with tc.tile_pool(name="ps_s", bufs=2, space="PSUM") as ps_s:
                    for kk in range(NKK):
                        exp_lo = explo_pool.tile([128, S // 2], fp16,
                                                 tag="exp_lo")
                        for qq in range(4):
                            ps = ps_s.tile([128, 1024], fp32, tag="ps")
                            for j in range(2):
                                nc.tensor.matmul(
                                    ps[:, j * 512:(j + 1) * 512],
                                    KT[:, kk * 128:(kk + 1) * 128],
                                    QT[:, qq * 1024 + j * 512:
                                       qq * 1024 + (j + 1) * 512],
                                    start=True, stop=True)
                            dst = (exp_lo[:, qq * 1024:(qq + 1) * 1024]
                                   if qq < 2 else
                                   expT_hi[:, kk, (qq - 2) * 1024:
                                           (qq - 1) * 1024])
                            nc.scalar.activation(
                                dst, ps[:],
                                mybir.ActivationFunctionType.Exp,
                                accum_out=sums[:, kk, qq:qq + 1])
                        nc.vector.reduce_sum(rsum[:, kk:kk + 1],
                                             sums[:, kk, :],
                                             axis=mybir.AxisListType.X)
                        nc.vector.reciprocal(rrec[:, kk:kk + 1],
                                             rsum[:, kk:kk + 1])
                        nc.vector.tensor_scalar_mul(Vs[:, kk, :], V[:, kk, :],
                                                    rrec[:, kk:kk + 1])
                        # PV software-pipelined one kk behind scores/exp.
                        if kk > 0:
                            pv_matmuls(kk - 1, po1, prev_lo[:], 0)
                        prev_lo = exp_lo
                    pv_matmuls(NKK - 1, po1, prev_lo[:], 0)
                drain_po(po1, 0)

            # ------------- Phase O2: PV for q-blocks 4..7 -----------------
            with tc.tile_pool(name="ps_o2", bufs=4, space="PSUM") as ps_o2:
                po2 = [ps_o2.tile([D, 512], fp32, tag="po2", name=f"po2_{i}") for i in range(4)]
                for kk in range(NKK):
                    pv_matmuls(kk, po2, expT_hi[:, kk, :], 0)
                drain_po(po2, 4)

    _dedup_ldweights(nc, mybir)
    nc.compile()
    return nc


def _dedup_ldweights(nc, mybir):
    """Drop InstLdweights that reload the weights already resident in the PE
    array (identical source AP as the previous load, with only
    non-self-loading matmuls in between). Tile's lowering emits one
    LDWEIGHTS per matmul; consecutive matmuls sharing a stationary operand
    only need the first."""
    remap = {}
    removed = 0
    for fn in nc.m.functions:
        for bb in fn.blocks:
            keep = []
            last_sig = None
            last_kept = None
            for inst in bb.instructions:
                if isinstance(inst, mybir.InstLdweights):
                    w = inst.ins[0]
                    try:
                        sig = (str(w.memref), str(w.memsetref), w.offset,
                               str(w.ap), str(w.dtype),
                               inst.perf_mode, inst.is_transpose)
                    except Exception:
                        sig = None
                    if sig is not None and last_kept is not None \
                            and sig == last_sig:
                        remap[inst.name] = last_kept.name
                        del nc.inst_map[inst.name]
                        removed += 1
                        continue
                    last_sig = sig
                    last_kept = inst
                elif isinstance(inst, mybir.InstMatmult):
                    if inst.ldweights is not False:
                        last_sig = None
                        last_kept = None
                keep.append(inst)
            if len(keep) != len(bb.instructions):
                bb.instructions[:] = keep
    if remap:
        for fn in nc.m.functions:
            for bb in fn.blocks:
                for inst in bb.instructions:
                    inst.remap_dependency_names(remap)
    return removed


def _get_module():
    if "nc" not in _CACHE:
        _CACHE["nc"] = _build_module()
    return _CACHE["nc"]


def kernel(x, Wq, bq, Wk, bk, Wv, bv, _trace=False):
    from concourse.bass_utils import run_bass_kernel_spmd

    x = np.asarray(x, dtype=np.float32)
    Wq = np.asarray(Wq, dtype=np.float32)
    bq = np.asarray(bq, dtype=np.float32)
    Wk = np.asarray(Wk, dtype=np.float32)
    bk = np.asarray(bk, dtype=np.float32)
    Wv = np.asarray(Wv, dtype=np.float32)
    bv = np.asarray(bv, dtype=np.float32)

    nc = _get_module()

    scale = np.float32(1.0 / np.sqrt(D))
    wq16 = (Wq * scale).astype(np.float16)
    wk16 = Wk.astype(np.float16)
    wv16 = Wv.astype(np.float16)
    bq_s = (bq * scale).astype(np.float32).reshape(D, 1)
    bk_s = bk.astype(np.float32).reshape(D, 1)
    bv_s = bv.astype(np.float32).reshape(D, 1)

    in_maps = []
    for c in range(N_CORES):
        b, kh = divmod(c, 2)
        xb = x[b]
        if kh:
            xb = np.concatenate([xb[SK:], xb[:SK]], axis=0)
        in_maps.append({
            "xT": np.ascontiguousarray(xb.T).astype(np.float16),
            "wq": wq16, "wk": wk16, "wv": wv16,
            "bq": bq_s, "bk": bk_s, "bv": bv_s,
        })

    res = run_bass_kernel_spmd(nc, in_maps,
                               core_ids=list(range(N_CORES)), trace=_trace)

    out = np.zeros((B, S, D), dtype=np.float32)
    for c in range(N_CORES):
        b, kh = divmod(c, 2)
        o = res.results[c]["outT"].T.astype(np.float32)  # [S, D], rolled q-order
        if kh:
            o = np.concatenate([o[SK:], o[:SK]], axis=0)
        out[b] += o
    if _trace:
        kernel.last_exec_time_ns = res.exec_time_ns
        kernel.last_result = res
    return out


# revision 12
# speedup vs baseline: 1.2745x; 1.2745x over previous
"""Trainium2 Bass kernel for single-head attention with softmax over the query axis.

Reference computation (B=4, S=4096, DIM=768, D=96):
    q = x @ Wq + bq; k = x @ Wk + bk; v = x @ Wv + bv        # [B,S,D]
    att = einsum('bqd,bkd->bqk', q, k) / sqrt(D)             # [B,Sq,Sk]
    p   = softmax(att, axis=1)                               # over the QUERY axis
    out = einsum('bqk,bkd->bqd', p, v)

Sharding: 8 cores = 4 batches x 2 key-halves. Softmax over q is local to a
key-shard (it normalizes each key-column over all queries), and the output
contraction over k is a sum over the two key-halves, done host-side.

SPMD uniformity trick: every core runs the identical program "K/V come from
rows 0:2048 of my x, Q from all 4096 rows". The host hands core (b, kh=1) a
row-rolled copy of x[b] so its key half lands in rows 0:2048; softmax over q
is permutation-invariant, and the host un-rolls that core's partial output.

Host precomputation (legal data prep inside kernel()): x is rolled,
transposed to xT [768, 4096] and cast to fp16; Wq/bq are pre-scaled by
1/sqrt(D) so no separate score scaling is needed; weights pre-cast to fp16.

On-device (per core):
  xT  [768, 4096] fp16 in SBUF (12 KB/partition)
  QT = Wq^T xT  [96, 4096], KT/VT likewise for rows 0:2048      (fp16)
  V[kk]  [128, 96] = PE-transpose of VT 128-column blocks        (fp16)
  scoresT[128k, q] = KT_kk^T QT in PSUM; exp on scalar engine with fused
      row-sum (accum_out); no max-subtraction (scores bounded ~|7.3|)
  Vs[kk] = V[kk] * (1/rowsum_kk)  folds softmax normalization into V
  outT[96, 4096] += Vs_kk^T @ expT_kk, accumulated in PSUM over kk;
      PV for q-blocks 0:2048 is software-pipelined inside the scores/exp
      loop (PSUM: 4 banks scores + 4 banks PV), the rest follows after.
"""

import os
import sys

import numpy as np

for _p in ("/opt/trn_rl_repo",):
    if _p not in sys.path and os.path.isdir(_p):
        sys.path.insert(0, _p)

B, S, DIM, D = 4, 4096, 768, 96
SK = S // 2          # local keys per core
N_CORES = 8
NDC = DIM // 128     # 6 dim chunks
NKK = SK // 128      # 16 local key chunks

_CACHE = {}


def _build_module():
    import concourse.bass as bass
    import concourse.tile as tile
    from concourse import bacc, mybir
    from concourse.masks import make_identity
    from concourse.tile import add_dep_helper

    fp32 = mybir.dt.float32
    fp16 = mybir.dt.float16

    nc = bacc.Bacc("TRN2", target_bir_lowering=False, debug=False,
                   num_devices=N_CORES)

    xT_ap = nc.dram_tensor("xT", [DIM, S], fp16, kind="ExternalInput").ap()
    wq_ap = nc.dram_tensor("wq", [DIM, D], fp16, kind="ExternalInput").ap()
    wk_ap = nc.dram_tensor("wk", [DIM, D], fp16, kind="ExternalInput").ap()
    wv_ap = nc.dram_tensor("wv", [DIM, D], fp16, kind="ExternalInput").ap()
    bq_ap = nc.dram_tensor("bq", [D, 1], fp32, kind="ExternalInput").ap()
    bk_ap = nc.dram_tensor("bk", [D, 1], fp32, kind="ExternalInput").ap()
    bv_ap = nc.dram_tensor("bv", [D, 1], fp32, kind="ExternalInput").ap()
    outT_ap = nc.dram_tensor("outT", [D, S], fp16, kind="ExternalOutput").ap()

    with tile.TileContext(nc) as tc:
        with (
            tc.tile_pool(name="singles", bufs=1) as singles,
            tc.tile_pool(name="acts", bufs=1) as acts,
            tc.tile_pool(name="outp", bufs=4) as outp,
        ):
            # Weights/biases first (tiny), then xT halves chained two-deep
            # so early chunks finish early (concurrent DMAs share bandwidth
            # equally; unchained, the first chunk lands no earlier than the
            # last and compute can't start for ~17us).
            w_sb = {}
            for name, ap in (("q", wq_ap), ("k", wk_ap), ("v", wv_ap)):
                w = singles.tile([128, NDC, D], fp16, tag=f"w{name}")
                nc.sync.dma_start(w[:], ap.rearrange("(c p) j -> p c j", p=128))
                w_sb[name] = w
            b_sb = {}
            for name, ap in (("q", bq_ap), ("k", bk_ap), ("v", bv_ap)):
                t = singles.tile([D, 1], fp32, tag=f"b{name}")
                nc.sync.dma_start(t[:], ap[:])
                b_sb[name] = t
            xTs = []
            for dc in range(NDC):
                t = singles.tile([128, S], fp16, tag=f"xT{dc}",
                                 name=f"xT{dc}")
                xTs.append(t)
            chain = []
            for half in range(2):
                lo, hi = (0, SK) if half == 0 else (SK, S)
                for dc in range(NDC):
                    h = nc.sync.dma_start(xTs[dc][:, lo:hi],
                                          xT_ap[dc * 128:(dc + 1) * 128,
                                                lo:hi])
                    chain.append(h)
            for i in range(4, len(chain)):
                add_dep_helper(chain[i].ins, chain[i - 4].ins,
                               reason="stagger xT DMAs: finish in issue order")
            identity = singles.tile([128, 128], fp16)
            make_identity(nc, identity[:])

            QT = acts.tile([D, S], fp16, tag="QT")
            KT = acts.tile([D, SK], fp16, tag="KT")
            VT = acts.tile([D, SK], fp16, tag="VT")
            V = acts.tile([128, NKK, D], fp16, tag="V")
            Vs = acts.tile([128, NKK, D], fp16, tag="Vs")
            sums = acts.tile([128, NKK, 4], fp32, tag="sums")
            rsum = acts.tile([128, NKK], fp32, tag="rsum")
            rrec = acts.tile([128, NKK], fp32, tag="rrec")
            # exp(scores) for q 0:2048 (written by S-A) stays resident for
            # the trailing PV pass; q 2048:4096 rotates through a small pool
            # consumed by the PV pipelined inside S-B.
            expT_A = acts.tile([128, NKK, S // 2], fp16, tag="expT_A")

            # ---------------- Phases -------------------------------------
            # P1: QT-lo (q 0:2048) and KT projections (only these gate the
            #     first scores).
            # S-A: scores+exp for q 0:2048 into the persistent expT_A, with
            #     the remaining projection work (QT-hi, VT, V-transposes)
            #     interleaved into the PE stream one unit per exp so it all
            #     hides under the scalar engine's exp pass.
            # S-B: scores+exp for q 2048:4096 into a rotating buffer,
            #     row-sums completed, Vs scaled, and PV for q 2048:4096
            #     software-pipelined one kk behind.
            # O2: PV for q 0:2048 from expT_A.
            def pv_matmuls(kk, po, src_tile, src_off):
                for qb in range(4):
                    nc.tensor.matmul(
                        po[qb][:], Vs[:, kk, :],
                        src_tile[:, src_off + qb * 512:
                                 src_off + (qb + 1) * 512],
                        start=(kk == 0), stop=(kk == NKK - 1))

            def drain_po(po, qb_base):
                for qb in range(4):
                    ob = outp.tile([D, 512], fp16, tag="ob")
                    nc.vector.tensor_copy(ob[:], po[qb][:])
                    nc.sync.dma_start(
                        outT_ap[:, (qb_base + qb) * 512:
                                (qb_base + qb + 1) * 512], ob[:])

            with tc.tile_pool(name="ps_s", bufs=2, space="PSUM") as ps_s:
                with tc.tile_pool(name="ps_proj", bufs=4,
                                  space="PSUM") as ps_proj:

                    def proj_mms(accs, wname, sb_base, nsb, dc):
                        for i in range(nsb):
                            sb = sb_base + i
                            nc.tensor.matmul(
                                accs[i][:], w_sb[wname][:, dc, :],
                                xTs[dc][:, sb * 512:(sb + 1) * 512],
                                start=(dc == 0), stop=(dc == NDC - 1))

                    def proj_drain(accs, dst, sb_base, nsb, bias):
                        for i in range(nsb):
                            sb = sb_base + i
                            nc.vector.tensor_scalar_add(
                                dst[:, sb * 512:(sb + 1) * 512],
                                accs[i][:], bias[:])

                    def alloc_accs(pfx, n=4):
                        return [ps_proj.tile([D, 512], fp32, tag="pp",
                                             name=f"{pfx}{i}")
                                for i in range(n)]

                    # P1: QT-lo and KT, sequential.
                    accs = alloc_accs("aql")
                    for dc in range(NDC):
                        proj_mms(accs, "q", 0, 4, dc)
                    proj_drain(accs, QT, 0, 4, b_sb["q"])
                    accs = alloc_accs("akt")
                    for dc in range(NDC):
                        proj_mms(accs, "k", 0, 4, dc)
                    proj_drain(accs, KT, 0, 4, b_sb["k"])

                    # Deferred P2 work, interleaved into S-A below.
                    p2_units = []
                    accs_qh = alloc_accs("aqh")
                    for dc in range(NDC):
                        p2_units.append(
                            lambda dc=dc: proj_mms(accs_qh, "q", 4, 4, dc))
                    p2_units.append(
                        lambda: proj_drain(accs_qh, QT, 4, 4, b_sb["q"]))
                    accs_vt = alloc_accs("avt")
                    for dc in range(NDC):
                        p2_units.append(
                            lambda dc=dc: proj_mms(accs_vt, "v", 0, 4, dc))
                    p2_units.append(
                        lambda: proj_drain(accs_vt, VT, 0, 4, b_sb["v"]))

                    def v_trans(kk):
                        pt = ps_proj.tile([128, D], fp16, tag="pp",
                                          name=f"pt{kk}")
                        nc.tensor.transpose(
                            pt[:], VT[:, kk * 128:(kk + 1) * 128],
                            identity[:D, :D])
                        nc.vector.tensor_copy(V[:, kk, :], pt[:])

                    for kk in range(NKK):
                        p2_units.append(lambda kk=kk: v_trans(kk))

                    # S-A: scores+exp for q 0:2048, one P2 unit per step.
                    ui = 0
                    for kk in range(NKK):
                        for qq in range(2):
                            ps = ps_s.tile([128, 1024], fp32, tag="ps")
                            for j in range(2):
                                nc.tensor.matmul(
                                    ps[:, j * 512:(j + 1) * 512],
                                    KT[:, kk * 128:(kk + 1) * 128],
                                    QT[:, qq * 1024 + j * 512:
                                       qq * 1024 + (j + 1) * 512],
                                    start=True, stop=True)
                            nc.scalar.activation(
                                expT_A[:, kk, qq * 1024:(qq + 1) * 1024],
                                ps[:], mybir.ActivationFunctionType.Exp,
                                accum_out=sums[:, kk, qq:qq + 1])
                            if ui < len(p2_units):
                                p2_units[ui]()
                                ui += 1
                    while ui < len(p2_units):
                        p2_units[ui]()
                        ui += 1

                # S-B: scores+exp for q 2048:4096 + pipelined PV(q hi half).
                with (
                    tc.tile_pool(name="ps_o1", bufs=4, space="PSUM") as ps_o1,
                    tc.tile_pool(name="exphi", bufs=2) as exphi_pool,
                ):
                    po1 = [ps_o1.tile([D, 512], fp32, tag="po",
                                      name=f"po1_{i}") for i in range(4)]
                    prev_hi = None
                    for kk in range(NKK):
                        exp_hi = exphi_pool.tile([128, S // 2], fp16,
                                                 tag="exp_hi")
                        for qq in (2, 3):
                            ps = ps_s.tile([128, 1024], fp32, tag="ps")
                            for j in range(2):
                                nc.tensor.matmul(
                                    ps[:, j * 512:(j + 1) * 512],
                                    KT[:, kk * 128:(kk + 1) * 128],
                                    QT[:, qq * 1024 + j * 512:
                                       qq * 1024 + (j + 1) * 512],
                                    start=True, stop=True)
                            nc.scalar.activation(
                                exp_hi[:, (qq - 2) * 1024:(qq - 1) * 1024],
                                ps[:], mybir.ActivationFunctionType.Exp,
                                accum_out=sums[:, kk, qq:qq + 1])
                        nc.vector.reduce_sum(rsum[:, kk:kk + 1],
                                             sums[:, kk, :],
                                             axis=mybir.AxisListType.X)
                        nc.vector.reciprocal(rrec[:, kk:kk + 1],
                                             rsum[:, kk:kk + 1])
                        nc.vector.tensor_scalar_mul(Vs[:, kk, :], V[:, kk, :],
                                                    rrec[:, kk:kk + 1])
                        if kk > 0:
                            pv_matmuls(kk - 1, po1, prev_hi[:], 0)
                        prev_hi = exp_hi
                    pv_matmuls(NKK - 1, po1, prev_hi[:], 0)
                    drain_po(po1, 4)

            # O2: PV for q 0:2048 from the persistent expT_A.
            with tc.tile_pool(name="ps_o2", bufs=4, space="PSUM") as ps_o2:
                po2 = [ps_o2.tile([D, 512], fp32, tag="po2",
                                  name=f"po2_{i}") for i in range(4)]
                for kk in range(NKK):
                    pv_matmuls(kk, po2, expT_A[:, kk, :], 0)
                drain_po(po2, 0)

    _dedup_ldweights(nc, mybir)
    nc.compile()
    return nc


def _dedup_ldweights(nc, mybir):
    """Drop InstLdweights that reload the weights already resident in the PE
    array (identical source AP as the previous load, with only
    non-self-loading matmuls in between). Tile's lowering emits one
    LDWEIGHTS per matmul; consecutive matmuls sharing a stationary operand
    only need the first."""
    remap = {}
    removed = 0
    for fn in nc.m.functions:
        for bb in fn.blocks:
            keep = []
            last_sig = None
            last_kept = None
            for inst in bb.instructions:
                if isinstance(inst, mybir.InstLdweights):
                    w = inst.ins[0]
                    try:
                        sig = (str(w.memref), str(w.memsetref), w.offset,
                               str(w.ap), str(w.dtype),
                               inst.perf_mode, inst.is_transpose)
                    except Exception:
                        sig = None
                    if sig is not None and last_kept is not None \
                            and sig == last_sig:
                        remap[inst.name] = last_kept.name
                        del nc.inst_map[inst.name]
                        removed += 1
                        continue
                    last_sig = sig
                    last_kept = inst
                elif isinstance(inst, mybir.InstMatmult):
                    if inst.ldweights is not False:
                        last_sig = None
                        last_kept = None
                keep.append(inst)
            if len(keep) != len(bb.instructions):
                bb.instructions[:] = keep
    if remap:
        for fn in nc.m.functions:
            for bb in fn.blocks:
                for inst in bb.instructions:
                    inst.remap_dependency_names(remap)
    return removed


def _get_module():
    if "nc" not in _CACHE:
        _CACHE["nc"] = _build_module()
    return _CACHE["nc"]


def kernel(x, Wq, bq, Wk, bk, Wv, bv, _trace=False):
    from concourse.bass_utils import run_bass_kernel_spmd

    x = np.asarray(x, dtype=np.float32)
    Wq = np.asarray(Wq, dtype=np.float32)
    bq = np.asarray(bq, dtype=np.float32)
    Wk = np.asarray(Wk, dtype=np.float32)
    bk = np.asarray(bk, dtype=np.float32)
    Wv = np.asarray(Wv, dtype=np.float32)
    bv = np.asarray(bv, dtype=np.float32)

    nc = _get_module()

    scale = np.float32(1.0 / np.sqrt(D))
    wq16 = (Wq * scale).astype(np.float16)
    wk16 = Wk.astype(np.float16)
    wv16 = Wv.astype(np.float16)
    bq_s = (bq * scale).astype(np.float32).reshape(D, 1)
    bk_s = bk.astype(np.float32).reshape(D, 1)
    bv_s = bv.astype(np.float32).reshape(D, 1)

    in_maps = []
    for c in range(N_CORES):
        b, kh = divmod(c, 2)
        xb = x[b]
        if kh:
            xb = np.concatenate([xb[SK:], xb[:SK]], axis=0)
        in_maps.append({
            "xT": np.ascontiguousarray(xb.T).astype(np.float16),
            "wq": wq16, "wk": wk16, "wv": wv16,
            "bq": bq_s, "bk": bk_s, "bv": bv_s,
        })

    res = run_bass_kernel_spmd(nc, in_maps,
                               core_ids=list(range(N_CORES)), trace=_trace)

    out = np.zeros((B, S, D), dtype=np.float32)
    for c in range(N_CORES):
        b, kh = divmod(c, 2)
        o = res.results[c]["outT"].T.astype(np.float32)  # [S, D], rolled q-order
        if kh:
            o = np.concatenate([o[SK:], o[:SK]], axis=0)
        out[b] += o
    if _trace:
        kernel.last_exec_time_ns = res.exec_time_ns
        kernel.last_result = res
    return out


# revision 13
# speedup vs baseline: 1.2880x; 1.0106x over previous
"""Trainium2 Bass kernel for single-head attention with softmax over the query axis.

Reference computation (B=4, S=4096, DIM=768, D=96):
    q = x @ Wq + bq; k = x @ Wk + bk; v = x @ Wv + bv        # [B,S,D]
    att = einsum('bqd,bkd->bqk', q, k) / sqrt(D)             # [B,Sq,Sk]
    p   = softmax(att, axis=1)                               # over the QUERY axis
    out = einsum('bqk,bkd->bqd', p, v)

Sharding: 8 cores = 4 batches x 2 key-halves. Softmax over q is local to a
key-shard (it normalizes each key-column over all queries), and the output
contraction over k is a sum over the two key-halves, done host-side.

SPMD uniformity trick: every core runs the identical program "K/V come from
rows 0:2048 of my x, Q from all 4096 rows". The host hands core (b, kh=1) a
row-rolled copy of x[b] so its key half lands in rows 0:2048; softmax over q
is permutation-invariant, and the host un-rolls that core's partial output.

Host precomputation (legal data prep inside kernel()): x is rolled,
transposed to xT [768, 4096] and cast to fp16; Wq/bq are pre-scaled by
1/sqrt(D) so no separate score scaling is needed; weights pre-cast to fp16.

On-device (per core):
  xT  [768, 4096] fp16 in SBUF (12 KB/partition)
  QT = Wq^T xT  [96, 4096], KT/VT likewise for rows 0:2048      (fp16)
  V[kk]  [128, 96] = PE-transpose of VT 128-column blocks        (fp16)
  scoresT[128k, q] = KT_kk^T QT in PSUM; exp on scalar engine with fused
      row-sum (accum_out); no max-subtraction (scores bounded ~|7.3|)
  Vs[kk] = V[kk] * (1/rowsum_kk)  folds softmax normalization into V
  outT[96, 4096] += Vs_kk^T @ expT_kk, accumulated in PSUM over kk;
      PV for q-blocks 0:2048 is software-pipelined inside the scores/exp
      loop (PSUM: 4 banks scores + 4 banks PV), the rest follows after.
"""

import os
import sys

import numpy as np

for _p in ("/opt/trn_rl_repo",):
    if _p not in sys.path and os.path.isdir(_p):
        sys.path.insert(0, _p)

B, S, DIM, D = 4, 4096, 768, 96
SK = S // 2          # local keys per core
N_CORES = 8
NDC = DIM // 128     # 6 dim chunks
NKK = SK // 128      # 16 local key chunks

_CACHE = {}


def _build_module():
    import concourse.bass as bass
    import concourse.tile as tile
    from concourse import bacc, mybir
    from concourse.masks import make_identity
    from concourse.tile import add_dep_helper

    fp32 = mybir.dt.float32
    fp16 = mybir.dt.float16

    nc = bacc.Bacc("TRN2", target_bir_lowering=False, debug=False,
                   num_devices=N_CORES)

    xT_ap = nc.dram_tensor("xT", [DIM, S], fp16, kind="ExternalInput").ap()
    wq_ap = nc.dram_tensor("wq", [DIM, D], fp16, kind="ExternalInput").ap()
    wk_ap = nc.dram_tensor("wk", [DIM, D], fp16, kind="ExternalInput").ap()
    wv_ap = nc.dram_tensor("wv", [DIM, D], fp16, kind="ExternalInput").ap()
    bq_ap = nc.dram_tensor("bq", [D, 1], fp32, kind="ExternalInput").ap()
    bk_ap = nc.dram_tensor("bk", [D, 1], fp32, kind="ExternalInput").ap()
    bv_ap = nc.dram_tensor("bv", [D, 1], fp32, kind="ExternalInput").ap()
    outT_ap = nc.dram_tensor("outT", [D, S], fp16, kind="ExternalOutput").ap()

    with tile.TileContext(nc) as tc:
        with (
            tc.tile_pool(name="singles", bufs=1) as singles,
            tc.tile_pool(name="acts", bufs=1) as acts,
            tc.tile_pool(name="outp", bufs=4) as outp,
        ):
            # Weights/biases first (tiny), then xT halves chained two-deep
            # so early chunks finish early (concurrent DMAs share bandwidth
            # equally; unchained, the first chunk lands no earlier than the
            # last and compute can't start for ~17us).
            w_sb = {}
            for name, ap in (("q", wq_ap), ("k", wk_ap), ("v", wv_ap)):
                w = singles.tile([128, NDC, D], fp16, tag=f"w{name}")
                nc.sync.dma_start(w[:], ap.rearrange("(c p) j -> p c j", p=128))
                w_sb[name] = w
            b_sb = {}
            for name, ap in (("q", bq_ap), ("k", bk_ap), ("v", bv_ap)):
                t = singles.tile([D, 1], fp32, tag=f"b{name}")
                nc.sync.dma_start(t[:], ap[:])
                b_sb[name] = t
            xTs = []
            for dc in range(NDC):
                t = singles.tile([128, S], fp16, tag=f"xT{dc}",
                                 name=f"xT{dc}")
                xTs.append(t)
            lo_dmas, hi_dmas = [], []
            for dc in range(NDC):
                h = nc.sync.dma_start(xTs[dc][:, :SK],
                                      xT_ap[dc * 128:(dc + 1) * 128, :SK])
                lo_dmas.append(h)
            for dc in range(NDC):
                h = nc.sync.dma_start(xTs[dc][:, SK:],
                                      xT_ap[dc * 128:(dc + 1) * 128, SK:])
                hi_dmas.append(h)
            for dc in range(NDC):
                add_dep_helper(hi_dmas[dc].ins, lo_dmas[dc].ins,
                               reason="xT hi halves yield bandwidth to lo")
            identity = singles.tile([128, 128], fp16)
            make_identity(nc, identity[:])

            QT = acts.tile([D, S], fp16, tag="QT")
            KT = acts.tile([D, SK], fp16, tag="KT")
            VT = acts.tile([D, SK], fp16, tag="VT")
            V = acts.tile([128, NKK, D], fp16, tag="V")
            Vs = acts.tile([128, NKK, D], fp16, tag="Vs")
            sums = acts.tile([128, NKK, 4], fp32, tag="sums")
            rsum = acts.tile([128, NKK], fp32, tag="rsum")
            rrec = acts.tile([128, NKK], fp32, tag="rrec")
            # exp(scores) for q 0:2048 (written by S-A) stays resident for
            # the trailing PV pass; q 2048:4096 rotates through a small pool
            # consumed by the PV pipelined inside S-B.
            expT_A = acts.tile([128, NKK, S // 2], fp16, tag="expT_A")

            # ---------------- Phases -------------------------------------
            # P1: QT-lo (q 0:2048) and KT projections (only these gate the
            #     first scores).
            # S-A: scores+exp for q 0:2048 into the persistent expT_A, with
            #     the remaining projection work (QT-hi, VT, V-transposes)
            #     interleaved into the PE stream one unit per exp so it all
            #     hides under the scalar engine's exp pass.
            # S-B: scores+exp for q 2048:4096 into a rotating buffer,
            #     row-sums completed, Vs scaled, and PV for q 2048:4096
            #     software-pipelined one kk behind.
            # O2: PV for q 0:2048 from expT_A.
            def pv_matmuls(kk, po, src_tile, src_off):
                for qb in range(4):
                    nc.tensor.matmul(
                        po[qb][:], Vs[:, kk, :],
                        src_tile[:, src_off + qb * 512:
                                 src_off + (qb + 1) * 512],
                        start=(kk == 0), stop=(kk == NKK - 1))

            def drain_po(po, qb_base):
                for qb in range(4):
                    ob = outp.tile([D, 512], fp16, tag="ob")
                    nc.vector.tensor_copy(ob[:], po[qb][:])
                    nc.sync.dma_start(
                        outT_ap[:, (qb_base + qb) * 512:
                                (qb_base + qb + 1) * 512], ob[:])

            with tc.tile_pool(name="ps_s", bufs=2, space="PSUM") as ps_s:
                with tc.tile_pool(name="ps_proj", bufs=4,
                                  space="PSUM") as ps_proj:

                    def proj_mms(accs, wname, sb_base, nsb, dc):
                        for i in range(nsb):
                            sb = sb_base + i
                            nc.tensor.matmul(
                                accs[i][:], w_sb[wname][:, dc, :],
                                xTs[dc][:, sb * 512:(sb + 1) * 512],
                                start=(dc == 0), stop=(dc == NDC - 1))

                    def proj_drain(accs, dst, sb_base, nsb, bias):
                        for i in range(nsb):
                            sb = sb_base + i
                            nc.vector.tensor_scalar_add(
                                dst[:, sb * 512:(sb + 1) * 512],
                                accs[i][:], bias[:])

                    def alloc_accs(pfx, n=4):
                        return [ps_proj.tile([D, 512], fp32, tag="pp",
                                             name=f"{pfx}{i}")
                                for i in range(n)]

                    # P1: QT-lo and KT, per 512-column block so the first
                    # scores can start as soon as QT sb0/sb1 + KT sb0 land.
                    for sb in range(4):
                        for wname, dst, bias in (("q", QT, b_sb["q"]),
                                                 ("k", KT, b_sb["k"])):
                            acc = ps_proj.tile([D, 512], fp32, tag="pp",
                                               name=f"a{wname}{sb}")
                            for dc in range(NDC):
                                nc.tensor.matmul(
                                    acc[:], w_sb[wname][:, dc, :],
                                    xTs[dc][:, sb * 512:(sb + 1) * 512],
                                    start=(dc == 0), stop=(dc == NDC - 1))
                            nc.vector.tensor_scalar_add(
                                dst[:, sb * 512:(sb + 1) * 512],
                                acc[:], bias[:])

                    # Deferred P2 work, interleaved into S-A below.
                    p2_units = []
                    accs_qh = alloc_accs("aqh")
                    for dc in range(NDC):
                        p2_units.append(
                            lambda dc=dc: proj_mms(accs_qh, "q", 4, 4, dc))
                    p2_units.append(
                        lambda: proj_drain(accs_qh, QT, 4, 4, b_sb["q"]))
                    accs_vt = alloc_accs("avt")
                    for dc in range(NDC):
                        p2_units.append(
                            lambda dc=dc: proj_mms(accs_vt, "v", 0, 4, dc))
                    p2_units.append(
                        lambda: proj_drain(accs_vt, VT, 0, 4, b_sb["v"]))

                    def v_trans(kk):
                        pt = ps_proj.tile([128, D], fp16, tag="pp",
                                          name=f"pt{kk}")
                        nc.tensor.transpose(
                            pt[:], VT[:, kk * 128:(kk + 1) * 128],
                            identity[:D, :D])
                        nc.vector.tensor_copy(V[:, kk, :], pt[:])

                    for kk in range(NKK):
                        p2_units.append(lambda kk=kk: v_trans(kk))

                    # S-A: scores+exp for q 0:2048, one P2 unit per step.
                    ui = 0
                    for kk in range(NKK):
                        for qq in range(2):
                            ps = ps_s.tile([128, 1024], fp32, tag="ps")
                            for j in range(2):
                                nc.tensor.matmul(
                                    ps[:, j * 512:(j + 1) * 512],
                                    KT[:, kk * 128:(kk + 1) * 128],
                                    QT[:, qq * 1024 + j * 512:
                                       qq * 1024 + (j + 1) * 512],
                                    start=True, stop=True)
                            nc.scalar.activation(
                                expT_A[:, kk, qq * 1024:(qq + 1) * 1024],
                                ps[:], mybir.ActivationFunctionType.Exp,
                                accum_out=sums[:, kk, qq:qq + 1])
                            if ui < len(p2_units):
                                p2_units[ui]()
                                ui += 1
                    while ui < len(p2_units):
                        p2_units[ui]()
                        ui += 1

                # S-B: scores+exp for q 2048:4096 + pipelined PV(q hi half).
                with (
                    tc.tile_pool(name="ps_o1", bufs=4, space="PSUM") as ps_o1,
                    tc.tile_pool(name="exphi", bufs=2) as exphi_pool,
                ):
                    po1 = [ps_o1.tile([D, 512], fp32, tag="po",
                                      name=f"po1_{i}") for i in range(4)]
                    prev_hi = None
                    for kk in range(NKK):
                        exp_hi = exphi_pool.tile([128, S // 2], fp16,
                                                 tag="exp_hi")
                        for qq in (2, 3):
                            ps = ps_s.tile([128, 1024], fp32, tag="ps")
                            for j in range(2):
                                nc.tensor.matmul(
                                    ps[:, j * 512:(j + 1) * 512],
                                    KT[:, kk * 128:(kk + 1) * 128],
                                    QT[:, qq * 1024 + j * 512:
                                       qq * 1024 + (j + 1) * 512],
                                    start=True, stop=True)
                            nc.scalar.activation(
                                exp_hi[:, (qq - 2) * 1024:(qq - 1) * 1024],
                                ps[:], mybir.ActivationFunctionType.Exp,
                                accum_out=sums[:, kk, qq:qq + 1])
                        nc.vector.reduce_sum(rsum[:, kk:kk + 1],
                                             sums[:, kk, :],
                                             axis=mybir.AxisListType.X)
                        nc.vector.reciprocal(rrec[:, kk:kk + 1],
                                             rsum[:, kk:kk + 1])
                        nc.vector.tensor_scalar_mul(Vs[:, kk, :], V[:, kk, :],
                                                    rrec[:, kk:kk + 1])
                        if kk > 0:
                            pv_matmuls(kk - 1, po1, prev_hi[:], 0)
                        prev_hi = exp_hi
                    pv_matmuls(NKK - 1, po1, prev_hi[:], 0)
                    drain_po(po1, 4)

            # O2: PV for q 0:2048 from the persistent expT_A.
            with tc.tile_pool(name="ps_o2", bufs=4, space="PSUM") as ps_o2:
                po2 = [ps_o2.tile([D, 512], fp32, tag="po2",
                                  name=f"po2_{i}") for i in range(4)]
                for kk in range(NKK):
                    pv_matmuls(kk, po2, expT_A[:, kk, :], 0)
                drain_po(po2, 0)

    _dedup_ldweights(nc, mybir)
    nc.compile()
    return nc


def _dedup_ldweights(nc, mybir):
    """Drop InstLdweights that reload the weights already resident in the PE
    array (identical source AP as the previous load, with only
    non-self-loading matmuls in between). Tile's lowering emits one
    LDWEIGHTS per matmul; consecutive matmuls sharing a stationary operand
    only need the first."""
    remap = {}
    removed = 0
    for fn in nc.m.functions:
        for bb in fn.blocks:
            keep = []
            last_sig = None
            last_kept = None
            for inst in bb.instructions:
                if isinstance(inst, mybir.InstLdweights):
                    w = inst.ins[0]
                    try:
                        sig = (str(w.memref), str(w.memsetref), w.offset,
                               str(w.ap), str(w.dtype),
                               inst.perf_mode, inst.is_transpose)
                    except Exception:
                        sig = None
                    if sig is not None and last_kept is not None \
                            and sig == last_sig:
                        remap[inst.name] = last_kept.name
                        del nc.inst_map[inst.name]
                        removed += 1
                        continue
                    last_sig = sig
                    last_kept = inst
                elif isinstance(inst, mybir.InstMatmult):
                    if inst.ldweights is not False:
                        last_sig = None
                        last_kept = None
                keep.append(inst)
            if len(keep) != len(bb.instructions):
                bb.instructions[:] = keep
    if remap:
        for fn in nc.m.functions:
            for bb in fn.blocks:
                for inst in bb.instructions:
                    inst.remap_dependency_names(remap)
    return removed


def _get_module():
    if "nc" not in _CACHE:
        _CACHE["nc"] = _build_module()
    return _CACHE["nc"]


def kernel(x, Wq, bq, Wk, bk, Wv, bv, _trace=False):
    from concourse.bass_utils import run_bass_kernel_spmd

    x = np.asarray(x, dtype=np.float32)
    Wq = np.asarray(Wq, dtype=np.float32)
    bq = np.asarray(bq, dtype=np.float32)
    Wk = np.asarray(Wk, dtype=np.float32)
    bk = np.asarray(bk, dtype=np.float32)
    Wv = np.asarray(Wv, dtype=np.float32)
    bv = np.asarray(bv, dtype=np.float32)

    nc = _get_module()

    scale = np.float32(1.0 / np.sqrt(D))
    wq16 = (Wq * scale).astype(np.float16)
    wk16 = Wk.astype(np.float16)
    wv16 = Wv.astype(np.float16)
    bq_s = (bq * scale).astype(np.float32).reshape(D, 1)
    bk_s = bk.astype(np.float32).reshape(D, 1)
    bv_s = bv.astype(np.float32).reshape(D, 1)

    in_maps = []
    for c in range(N_CORES):
        b, kh = divmod(c, 2)
        xb = x[b]
        if kh:
            xb = np.concatenate([xb[SK:], xb[:SK]], axis=0)
        in_maps.append({
            "xT": np.ascontiguousarray(xb.T).astype(np.float16),
            "wq": wq16, "wk": wk16, "wv": wv16,
            "bq": bq_s, "bk": bk_s, "bv": bv_s,
        })

    res = run_bass_kernel_spmd(nc, in_maps,
                               core_ids=list(range(N_CORES)), trace=_trace)

    out = np.zeros((B, S, D), dtype=np.float32)
    for c in range(N_CORES):
        b, kh = divmod(c, 2)
        o = res.results[c]["outT"].T.astype(np.float32)  # [S, D], rolled q-order
        if kh:
            o = np.concatenate([o[SK:], o[:SK]], axis=0)
        out[b] += o
    if _trace:
        kernel.last_exec_time_ns = res.exec_time_ns
        kernel.last_result = res
    return out
